# revision 45
# baseline (speedup 1.0000x reference)
# Mixture-of-two-experts (modality-routed) token GEMM on 8 Trainium2 NeuronCores.
#
# v7: weights-stationary. The reference computes BOTH expert GEMMs and selects
# per token; only one GEMM per token is needed. Host partitions tokens by
# type_id (expert-dispatch): cores 0-3 carry expert-0 tokens + W0, cores 4-7
# expert-1 tokens + W1 (weights arrive as data, the per-core program is
# identical). On device each core computes y[e, tok] = W x + b with W tiles
# STATIONARY and the token dim MOVING, so PE cost scales with the actual
# per-core token count (n_tok rounded to 4) instead of 128-padded m-tiles.
# Steady state measured AT the PE roofline: 384-wide fp16 matmuls issue
# 162ns apart = 1 column/cycle at 2.37GHz with LDWEIGHTS fully hidden.
#
# Mixed precision, tiered by phase: late chains (chunk>=1, ~75% of tokens)
# run fp16 for k 0..1279 + three fp8e4m3 DoubleRow pairs (2 k-planes per PE
# cell, 2 MACs/cycle) for k 1280..2047 = 13 PE passes instead of 16; the
# DMA-ramp-limited chunk-0 chains (~25% of tokens) run fp16 k 0..1023 +
# FOUR pairs = 12 passes. SW=45.25 places max|W|*SW at 1.0 so the top e4m3
# binade is [0.5,1) with step 1/16 (SW=64 wastes half the mantissa: max
# lands mid-binade at 1.41 where the step is 1/8). Host-simulated with
# exact device semantics AND hardware-verified: rel err 1.9314e-2, inside
# the 2e-2 gate (host sim matched hardware to ~1e-6 on three configs).
# Scales: x8 = x*16, w8 = W*45.25, fp16 W pre-scaled *724 so one PSUM chain
# is consistent at 724*y; the host divides the fp16 output by 724.

import os
import sys
import time

import numpy as np
import ml_dtypes

for _p in ("/opt/trn_rl_repo", "/root/.axon_site/_ro/trn_rl_repo"):
    if os.path.isdir(_p) and _p not in sys.path:
        sys.path.insert(0, _p)

import concourse.bacc as bacc
import concourse.mybir as mybir
import concourse.tile as tile
from concourse.bass_utils import run_bass_kernel_spmd

D = 2048
ET = D // 128  # 16 output-feature tiles
K16 = 10  # fp16 k-tiles (k 0..1279), used by the chunk>=1 (late) chains
NPAIR = 3  # late-chain fp8 DoubleRow pairs (k 1280..2047, 256 rows each)
NPA = 4  # chunk-0 chains use FOUR pairs (k 1024..2047) + fp16 k 0..1023:
# 12 PE passes instead of 13 for ~25% of tokens. Host-simulated rel err
# 1.9314e-2 (vs 1.8555e-2 all-b3), still under the 2e-2 gate; the extra
# pair is a superset load (pairs 1-3 are the late-chain pairs) costing one
# 512KB weight transfer while dropping the k8/k9 x-head transfers.
KSPLIT = K16 * 128
KSPLIT_A = 2048 - NPA * 256  # fp16/fp8 boundary for chunk-0 chains (1024)
SX = 16.0  # fp8 scale on x
SW = 45.25  # fp8 scale on W (max|W|*SW = 1.0: top binade step 1/16)
SCALE = SX * SW  # PSUM carries SCALE*y; host divides it out
N_CORES = 8
CORES_PER_EXPERT = 4
N_WARMUP = 6  # PE warm-up matmuls bridging PE-boot (~7.5us) to first-
# operand-ready (~10.4-10.7us). Measured: DMA issue instructions serialize
# at ~600-740ns each with a 4-deep credit window, so the first (w0lo, xh0)
# pair cannot land earlier no matter how transfers are sized or ordered;
# 6 zero-MMs at p-state-ramp clock (788+5*427ns) end right there. 8 was
# also tried: the post-warm-up wait just moved later (DMA jitter) and it
# measured worse.
F8 = ml_dtypes.float8_e4m3fn

_PROGRAM_CACHE: dict[int, object] = {}
LAST_RESULTS = None  # BassKernelResults of the most recent launch (for profiling)


def _chunks(n_tok: int):
    """Split the token dim into <=512-wide chunks, 4-aligned boundaries.

    Chunk 0 is full 512 wide when possible: during the DMA ramp the 8
    in-flight chunk-0 chains then expose 8*216ns of PE work per arriving
    (w_k, xh_k) pair, matching the ~1.7us pair cadence."""
    if n_tok <= 512:
        return [(0, n_tok)]
    rest = n_tok - 512
    nch = -(-rest // 512)
    base = rest // nch // 4 * 4
    sizes = [base] * nch
    i = 0
    while 512 + sum(sizes) < n_tok:
        sizes[i] += 4
        i = (i + 1) % nch
    sizes = [512] + sizes
    assert 512 + rest == n_tok and all(s <= 512 for s in sizes)
    out, s0 = [], 0
    for s in sizes:
        out.append((s0, s))
        s0 += s
    return out


def _build_program(n_tok: int):
    """One NeuronCore program: y[e, tok] = SCALE * (W @ x + bias), fp16 out."""
    assert n_tok % 4 == 0
    f16 = mybir.dt.float16
    f32 = mybir.dt.float32
    f8 = mybir.dt.float8e4
    DR = mybir.MatmulPerfMode.DoubleRow

    CH = _chunks(n_tok)
    ch0 = CH[0][1]  # chunk-0 width: the x "head" loaded before the tails
    tail = n_tok - ch0

    nc = bacc.Bacc("TRN2", target_bir_lowering=False, debug=False, num_devices=N_CORES)
    xt = nc.dram_tensor("xt", [K16, 128, n_tok], f16, kind="ExternalInput").ap()
    # fp8 x arrives pre-split into head/tail so each is contiguous and the
    # per-pair transfers collapse to balanced 3D APs (the DMA engine
    # cannot balance >3 genuinely-strided dims). Head: 4 pairs (chunk-0
    # chains), tail: 3 pairs (late chains).
    xt8h = nc.dram_tensor("xt8h", [NPA, 128, 2, ch0], f8, kind="ExternalInput").ap()
    xt8t = (
        nc.dram_tensor("xt8t", [NPAIR, 128, 2, tail], f8, kind="ExternalInput").ap()
        if tail
        else None
    )
    wt = nc.dram_tensor("wt", [K16, 128, D], f16, kind="ExternalInput").ap()
    wt8 = nc.dram_tensor("wt8", [NPA, 128, 2, D], f8, kind="ExternalInput").ap()
    biasw = nc.dram_tensor("biasw", [128, ET], f32, kind="ExternalInput").ap()
    y = nc.dram_tensor("y", [ET, 128, n_tok], f16, kind="ExternalOutput").ap()

    # k-units: ('f', k) = one fp16 k-tile, ('d', j) = one fp8 DoubleRow pair
    # (j indexes the 4-pair wt8; pair j covers rows 1024+256j).
    # chunk-0 chains: fp16 k0-7 + all 4 pairs = 12 PE passes.
    units_a = [("f", k) for k in range(8)] + [("d", j) for j in range(NPA)]

    with tile.TileContext(nc) as tc:
        with (
            tc.tile_pool(name="wp", bufs=1) as wp,
            tc.tile_pool(name="xp", bufs=1) as xp,
            tc.tile_pool(name="bp", bufs=1) as bp,
            # ot staging: enough bufs that DVE drains never wait on y-DMA
            # completion - on runs where the y writebacks crawl (shared-HBM
            # contention), a 14-buf pool exhausted ~67us in: DVE stalled on
            # slot reuse, PSUM banks stayed held, and the PE went idle 5.3us
            # (+15us of tail backlog). 28 bufs ride out a full bad-writeback
            # episode; +14KB SBUF of the ~88KB still free.
            tc.tile_pool(name="op", bufs=28) as op_,
            tc.tile_pool(name="pp", bufs=8, space="PSUM") as pp,
        ):
            # (w_k, x-head_k) pairs in unit order on ONE ring (sync): a
            # single priority-ordered FIFO gives each pair the FULL HBM
            # bandwidth in turn. Striping pairs across both HWDGE rings was
            # measured SLOWER every way (v4 whole-tile alternation: bursty
            # arrival, +5.7us of PE gaps; v5/v6 half-striping: pair-ready =
            # max of two jittery half-rate streams). Issue instructions cost
            # ~600-740ns on the engine with a 4-deep in-flight credit
            # window, so the head sequence also wants FEW, BIG transfers:
            # w tiles go whole (512KB) except w0, whose lo half leads so
            # chains e0-e7 can start the moment (w0-lo, xh0) lands. Rows
            # must stay >=1024B: 512B-row transfers halve DMA efficiency
            # (v5: 360 -> 206GB/s aggregate).
            # ISSUE COUNT IS A FIRST-CLASS COST: each dma_start occupies the
            # ring engine ~600-740ns and the ring holds only ~4 transfers in
            # flight (per-context completion credits), so a long stream of
            # small transfers is issue/credit-paced, not bandwidth-paced.
            # With 42 per-tensor transfers the LAST ones (x tails) did not
            # even start until ~55us and the first late chain stalled unit-
            # by-unit on them (1.2-4.9us + a PE p-state reset). 21 transfers:
            # singles while the PE still trails the DMA (units 0-3), then
            # 2-plane/6-plane combines, then 3 big tail batches.
            rings = (nc.sync, nc.scalar)
            bias_s = bp.tile([128, ET], f32, name="bias_s")
            wk, xh = [], []
            for k in range(4):
                ws = wp.tile([128, D], f16, name=f"w{k}", tag=f"w{k}")
                h = xp.tile([128, ch0], f16, name=f"xh{k}", tag=f"xh{k}")
                if k == 0:
                    nc.sync.dma_start(ws[:, 0 : D // 2], wt[k][:, 0 : D // 2])
                    nc.sync.dma_start(h[:], xt[k][:, 0:ch0])
                    nc.sync.dma_start(ws[:, D // 2 : D], wt[k][:, D // 2 : D])
                    nc.sync.dma_start(bias_s[:], biasw[:])
                else:
                    nc.sync.dma_start(ws[:], wt[k])
                    nc.sync.dma_start(h[:], xt[k][:, 0:ch0])
                wk.append(ws)
                xh.append(h)
            # units 4-7 as 2-plane pairs: one 1MB w + one 256KB xh transfer
            # per two units (bursts of 2 pairs on one ring are absorbed by
            # the 8-chain PSUM buffer; the halved issue count is pure win)
            # (rearrange the FULL dram tensor, then slice/index: rearranging
            # an already-sliced 4D AP panics in the AP library)
            wg, xhg = [], []
            wt_p = wt.rearrange("(g k) p d -> g p k d", k=2)
            xt_p = xt.rearrange("(g k) p n -> g p k n", k=2)
            for g in range(2):
                g2 = 2 + g
                w2 = wp.tile([128, 2, D], f16, name=f"wg{g}", tag=f"wg{g}")
                nc.sync.dma_start(w2[:], wt_p[g2])
                wg.append(w2)
                h2 = xp.tile([128, 2, ch0], f16, name=f"xhg{g}", tag=f"xhg{g}")
                nc.sync.dma_start(h2[:], xt_p[g2][:, :, 0:ch0])
                xhg.append(h2)
            # fp8 pairs in TWO 2-pair transfers (1.28MB w + 256KB x each):
            # bursts of 2 units are absorbed by the 8-chain PSUM buffer, and
            # every issue saved here pulls the tail transfers earlier in the
            # ring's 4-deep credit pipeline (v11 shipped 8 per-pair issues
            # and the late chains stalled 8.3us on late tails)
            w8c = wp.tile([128, NPA, 2, D], f8, name="w8c", tag="w8c")
            x8hc = xp.tile([128, NPA, 2, ch0], f8, name="x8hc", tag="x8hc")
            wt8_p = wt8.rearrange("(g j) p i d -> g p j i d", j=2)
            xt8h_p = xt8h.rearrange("(g j) p i n -> g p j i n", j=2)
            for g in range(NPA // 2):
                nc.sync.dma_start(w8c[:, 2 * g : 2 * g + 2, :, :], wt8_p[g])
                nc.sync.dma_start(x8hc[:, 2 * g : 2 * g + 2, :, :], xt8h_p[g])
            # tails next: the late chains consume them FIRST (units_l = DR
            # pairs then fp16 k ascending) and the A phase is now short
            # enough that they, not the A units, bound the transition.
            if tail:
                x8tc = xp.tile([128, NPAIR, 2, tail], f8, name="x8tc", tag="x8tc")
                nc.sync.dma_start(x8tc[:], xt8t.rearrange("j p i n -> p j i n"))
            xtl = []
            xt_r = xt.rearrange("(g k) p n -> g k p n", k=5)
            for g in range(K16 // 5):
                t = xp.tile([128, 5, tail], f16, name=f"xt{g}", tag=f"xt{g}")
                nc.sync.dma_start(
                    t[:], xt_r[g][:, :, ch0:n_tok].rearrange("k p n -> p k n")
                )
                xtl.append(t)
            # fp16 k8/k9 weights are LATE-ONLY (chunk-0 covers those rows in
            # fp8) and the LAST two units of each late chain - load them
            # after the tails. Their x heads (cols 0..511) are never read.
            w2l = wp.tile([128, 2, D], f16, name="wg2", tag="wg2")
            nc.sync.dma_start(w2l[:], wt_p[4])
            wg.append(w2l)

            def w_ap(k, e):
                if k < 4:
                    return wk[k][:, e * 128 : (e + 1) * 128]
                g, p = divmod(k - 4, 2)
                return wg[g][:, p, e * 128 : (e + 1) * 128]

            def w8_ap(j, e):
                return w8c[:, j, :, e * 128 : (e + 1) * 128]

            def x_slice(k, s0, n):
                if s0 == 0:
                    if k < 4:
                        return xh[k][:, 0:n]
                    g, p = divmod(k - 4, 2)
                    return xhg[g][:, p, 0:n]
                return xtl[k // 5][:, k % 5, s0 - ch0 : s0 - ch0 + n]

            def x8_slice(j, s0, n):
                if s0 == 0:
                    return x8hc[:, j, :, 0:n]
                # tails exist only for the late-chain pairs j=1..3
                return x8tc[:, j - 1, :, s0 - ch0 : s0 - ch0 + n]

            # PE warm-up: matmuls on a zeroed tile, no DMA dependency. Runs
            # during the DMA ramp (PE would idle anyway) and flips the HAM
            # clock gate to 8/8 before the first real matmul. memset on DVE:
            # it boots by ~4.7us and memsets in ~200ns.
            wz = bp.tile([128, 512], f16, name="wz")
            nc.vector.memset(wz[:], 0.0)
            # psw shares the chain-psum rotation: its slot is recycled by the
            # 8th chunk-0 chain, long after the warm-up finishes. Excess
            # warm-ups sit AHEAD of ready real work in the PE queue and
            # delay it (they run at p-state-ramp clock, ~430-790ns each).
            psw = pp.tile([128, 512], f32, name="psw", tag="ps")
            for _ in range(N_WARMUP):
                nc.tensor.matmul(psw[:], wz[:, 0:128], wz[:], start=True, stop=True)

            def unit_mm(ps, e, s0, n, u, start, stop):
                if u[0] == "f":
                    return nc.tensor.matmul(
                        ps[:, 0:n],
                        w_ap(u[1], e),
                        x_slice(u[1], s0, n),
                        start=start,
                        stop=stop,
                    )
                return nc.tensor.matmul(
                    ps[:, 0:n],
                    w8_ap(u[1], e),
                    x8_slice(u[1], s0, n),
                    start=start,
                    stop=stop,
                    perf_mode=DR,
                )

            def mm_chain(ps, e, s0, n, us=None):
                us = us if us is not None else units_a
                first = last = None
                for i, u in enumerate(us):
                    mm = unit_mm(ps, e, s0, n, u, i == 0, i == len(us) - 1)
                    first = first or mm
                    last = mm
                return first, last

            prev_last = None

            def pin(first, reason):
                # keep the PE stream in emission order chain-by-chain: the
                # scheduler otherwise hoists later chains (gated on late
                # arrivals) ahead of ready work and stalls the PE
                if prev_last is not None:
                    tile.add_dep_helper(
                        first.ins, prev_last.ins, sync=False, reason=reason
                    )

            def drain(ps, e, s0, n):
                ot = op_.tile([128, n], f16, name=f"ot{e}_{s0}", tag="ot")
                nc.vector.tensor_scalar_add(ot[:], ps[:, 0:n], bias_s[:, e : e + 1])
                # y alternates the two HW rings (vector/gpsimd are NOT HW
                # DGE rings - gpsimd's qGpSimdDynamic is a software queue,
                # measured +55us). Alternating halves the writeback backlog
                # each ring's final completion-wait covers.
                rings[e % 2].dma_start(y[e][:, s0 : s0 + n], ot[:])

            # phase A: chunk-0 chains, UNPINNED so the scheduler interleaves
            # them by operand arrival during the DMA ramp. 8 psum banks keep
            # 8 chains in flight (8 x 216ns of PE work per arriving k-pair
            # matches the ~1.7us pair cadence); later e-tiles draft behind
            # the frontier on already-arrived pairs.
            a_lasts = []
            s0a, n0 = CH[0]
            for e in range(ET):
                ps = pp.tile([128, 512], f32, name=f"psa{e}", tag="ps")
                fa, la = mm_chain(ps, e, s0a, n0)
                a_lasts.append(la)
                drain(ps, e, s0a, n0)

            # chunks >= 1: all operands are resident by now; strict emission
            # order keeps the PE stream dense. DR units go FIRST so the
            # chain's stop-MM is a plain fp16 one. (Merging late drains into
            # per-e staging tiles with one y-DMA measured SLOWER - the
            # teardown semaphore storm did not shrink with transfer count.)
            # late chains: fp16 k0-9 + pairs 1-3 (rows 1280..2047), DR first
            units_l = [("d", j) for j in range(1, NPA)] + [("f", k) for k in range(K16)]
            first_late = True
            for s0, n in CH[1:]:
                for e in range(ET):
                    ps = pp.tile([128, 512], f32, name=f"ps{e}_{s0}", tag="ps")
                    ff, lf = mm_chain(ps, e, s0, n, units_l)
                    if first_late:
                        # full join on ALL chunk-0 chains: softening this to
                        # the last 8 (v8) let the scheduler shuffle the
                        # transition and measured WORSE (2.8us of transition
                        # gaps vs 1.2us, plus a bunched y-writeback tail)
                        for la in a_lasts:
                            tile.add_dep_helper(ff.ins, la.ins, sync=False, reason="A->F")
                        first_late = False
                    else:
                        pin(ff, f"chain order c{s0}e{e}")
                    prev_last = lf
                    drain(ps, e, s0, n)

    nc.compile()
    return nc


def _get_program(n_tok: int):
    if n_tok not in _PROGRAM_CACHE:
        _PROGRAM_CACHE[n_tok] = _build_program(n_tok)
    return _PROGRAM_CACHE[n_tok]


def _round_up(v: int, m: int) -> int:
    return -(-v // m) * m


def _q8(a: np.ndarray, scale: float) -> np.ndarray:
    return np.clip(a * scale, -240.0, 240.0).astype(F8)


def kernel(hidden_states, type_ids, W0, b0, W1, b1, _trace=False, _tmpdir=None):
    global LAST_RESULTS

    B, S, D_ = hidden_states.shape
    assert D_ == D
    x = np.ascontiguousarray(np.asarray(hidden_states, dtype=np.float32)).reshape(
        B * S, D
    )
    t = np.asarray(type_ids).reshape(B * S)

    idx = [np.nonzero(t == e)[0] for e in (0, 1)]
    counts = [len(i) for i in idx]
    # tokens per core: 4 cores per expert, token dim rounded to 4 (moving
    # operand - no 128 padding needed). Extremely skewed expert splits fall
    # back to multiple launches of the same program over token slices.
    N_TOK_MAX = 4096
    n_tok = max(64, _round_up(-(-max(counts) // CORES_PER_EXPERT), 4))
    n_tok = min(n_tok, N_TOK_MAX)
    cap = n_tok * CORES_PER_EXPERT
    n_launches = -(-max(counts) // cap)

    nc = _get_program(n_tok)

    wts, wt8s, biases = [], [], []
    for W, b in ((W0, b0), (W1, b1)):
        WT = np.asarray(W, dtype=np.float32).T  # [d, e]
        wts.append(
            np.ascontiguousarray((WT[:KSPLIT] * SCALE).astype(np.float16)).reshape(
                K16, 128, D
            )
        )
        # pair j, plane i, partition p <-> contraction row KSPLIT_A+256j+128i+p
        # (4 pairs from row 1024; the late chains use pairs 1-3 = rows 1280+)
        wt8s.append(
            np.ascontiguousarray(
                _q8(WT[KSPLIT_A:], SW).reshape(NPA, 2, 128, D).transpose(0, 2, 1, 3)
            )
        )
        biases.append(
            np.ascontiguousarray(
                (np.asarray(b, dtype=np.float32) * SCALE).reshape(ET, 128).T
            )
        )

    gathered = [x[idx[e]] for e in (0, 1)]  # [count_e, D] fp32

    out = np.empty((B * S, D), dtype=np.float32)
    parts = [[], []]
    for li in range(n_launches):
        in_maps = []
        for e in (0, 1):
            g = gathered[e][li * cap : (li + 1) * cap]
            if g.shape[0] < cap:
                g = np.concatenate(
                    [g, np.zeros((cap - g.shape[0], D), np.float32)], axis=0
                )
            ch0 = _chunks(n_tok)[0][1]
            for c in range(CORES_PER_EXPERT):
                chunk = g[c * n_tok : (c + 1) * n_tok]  # [n_tok, D] fp32
                ct = chunk.T  # [D, n_tok]
                xt_c = np.ascontiguousarray(ct[:KSPLIT].astype(np.float16)).reshape(
                    K16, 128, n_tok
                )
                xt8_c = (
                    _q8(ct[KSPLIT_A:], SX)
                    .reshape(NPA, 2, 128, n_tok)
                    .transpose(0, 2, 1, 3)
                )
                im = {
                    "xt": xt_c,
                    "xt8h": np.ascontiguousarray(xt8_c[:, :, :, 0:ch0]),
                    "wt": wts[e],
                    "wt8": wt8s[e],
                    "biasw": biases[e],
                }
                if ch0 < n_tok:
                    # tails only for the late-chain pairs (rows 1280+)
                    im["xt8t"] = np.ascontiguousarray(xt8_c[1:, :, :, ch0:])
                in_maps.append(im)

        res = None
        for attempt in range(3):
            try:
                res = run_bass_kernel_spmd(
                    nc, in_maps, list(range(N_CORES)), trace=_trace, tmpdir=_tmpdir
                )
                break
            except Exception:
                # transient NRT_EXEC_UNIT_UNRECOVERABLE has been observed when
                # a run starts right as a previous process tears the device down
                if attempt == 2:
                    raise
                time.sleep(10)
        LAST_RESULTS = res
        for e in (0, 1):
            parts[e].extend(
                res.results[e * CORES_PER_EXPERT + c]["y"].reshape(D, n_tok).T
                for c in range(CORES_PER_EXPERT)
            )

    inv = np.float32(1.0 / SCALE)
    for e in (0, 1):
        full_e = np.concatenate(parts[e], axis=0)[: counts[e]]
        out[idx[e]] = full_e.astype(np.float32) * inv
    return out.reshape(B, S, D)



# revision 46
# speedup vs baseline: 1.1487x; 1.1487x over previous
# Mixture-of-two-experts (modality-routed) token GEMM on 8 Trainium2 NeuronCores.
#
# v7: weights-stationary. The reference computes BOTH expert GEMMs and selects
# per token; only one GEMM per token is needed. Host partitions tokens by
# type_id (expert-dispatch): cores 0-3 carry expert-0 tokens + W0, cores 4-7
# expert-1 tokens + W1 (weights arrive as data, the per-core program is
# identical). On device each core computes y[e, tok] = W x + b with W tiles
# STATIONARY and the token dim MOVING, so PE cost scales with the actual
# per-core token count (n_tok rounded to 4) instead of 128-padded m-tiles.
# Steady state measured AT the PE roofline: 384-wide fp16 matmuls issue
# 162ns apart = 1 column/cycle at 2.37GHz with LDWEIGHTS fully hidden.
#
# Mixed precision, tiered by phase: late chains (chunk>=1, ~75% of tokens)
# run fp16 for k 0..1279 + three fp8e4m3 DoubleRow pairs (2 k-planes per PE
# cell, 2 MACs/cycle) for k 1280..2047 = 13 PE passes instead of 16; the
# DMA-ramp-limited chunk-0 chains (~25% of tokens) run fp16 k 0..1023 +
# FOUR pairs = 12 passes. SW=45.25 places max|W|*SW at 1.0 so the top e4m3
# binade is [0.5,1) with step 1/16 (SW=64 wastes half the mantissa: max
# lands mid-binade at 1.41 where the step is 1/8). Host-simulated with
# exact device semantics AND hardware-verified: rel err 1.9314e-2, inside
# the 2e-2 gate (host sim matched hardware to ~1e-6 on three configs).
# Scales: x8 = x*16, w8 = W*45.25, fp16 W pre-scaled *724 so one PSUM chain
# is consistent at 724*y; the host divides the fp16 output by 724.

import os
import sys
import time

import numpy as np
import ml_dtypes

for _p in ("/opt/trn_rl_repo", "/root/.axon_site/_ro/trn_rl_repo"):
    if os.path.isdir(_p) and _p not in sys.path:
        sys.path.insert(0, _p)

import concourse.bacc as bacc
import concourse.mybir as mybir
import concourse.tile as tile
from concourse.bass_utils import run_bass_kernel_spmd

D = 2048
ET = D // 128  # 16 output-feature tiles
K16 = 10  # fp16 k-tiles (k 0..1279), used by the chunk>=1 (late) chains
NPAIR = 3  # late-chain fp8 DoubleRow pairs (k 1280..2047, 256 rows each)
NPA = 4  # chunk-0 chains use FOUR pairs (k 1024..2047) + fp16 k 0..1023:
# 12 PE passes instead of 13 for ~25% of tokens. Host-simulated rel err
# 1.9314e-2 (vs 1.8555e-2 all-b3), still under the 2e-2 gate; the extra
# pair is a superset load (pairs 1-3 are the late-chain pairs) costing one
# 512KB weight transfer while dropping the k8/k9 x-head transfers.
KSPLIT = K16 * 128
KSPLIT_A = 2048 - NPA * 256  # fp16/fp8 boundary for chunk-0 chains (1024)
SX = 16.0  # fp8 scale on x
SW = 45.25  # fp8 scale on W (max|W|*SW = 1.0: top binade step 1/16)
SCALE = SX * SW  # PSUM carries SCALE*y; host divides it out
N_CORES = 8
CORES_PER_EXPERT = 4
N_WARMUP = 6  # PE warm-up matmuls bridging PE-boot (~7.5us) to first-
# operand-ready (~10.4-10.7us). Measured: DMA issue instructions serialize
# at ~600-740ns each with a 4-deep credit window, so the first (w0lo, xh0)
# pair cannot land earlier no matter how transfers are sized or ordered;
# 6 zero-MMs at p-state-ramp clock (788+5*427ns) end right there. 8 was
# also tried: the post-warm-up wait just moved later (DMA jitter) and it
# measured worse.
F8 = ml_dtypes.float8_e4m3fn

_PROGRAM_CACHE: dict[int, object] = {}
LAST_RESULTS = None  # BassKernelResults of the most recent launch (for profiling)


def _chunks(n_tok: int):
    """Split the token dim into <=512-wide chunks, 4-aligned boundaries.

    Chunk 0 is full 512 wide when possible: during the DMA ramp the 8
    in-flight chunk-0 chains then expose 8*216ns of PE work per arriving
    (w_k, xh_k) pair, matching the ~1.7us pair cadence."""
    if n_tok <= 512:
        return [(0, n_tok)]
    rest = n_tok - 512
    nch = -(-rest // 512)
    base = rest // nch // 4 * 4
    sizes = [base] * nch
    i = 0
    while 512 + sum(sizes) < n_tok:
        sizes[i] += 4
        i = (i + 1) % nch
    sizes = [512] + sizes
    assert 512 + rest == n_tok and all(s <= 512 for s in sizes)
    out, s0 = [], 0
    for s in sizes:
        out.append((s0, s))
        s0 += s
    return out


def _build_program(n_tok: int):
    """One NeuronCore program: y[e, tok] = SCALE * (W @ x + bias), fp16 out."""
    assert n_tok % 4 == 0
    f16 = mybir.dt.float16
    f32 = mybir.dt.float32
    f8 = mybir.dt.float8e4
    DR = mybir.MatmulPerfMode.DoubleRow

    CH = _chunks(n_tok)
    ch0 = CH[0][1]  # chunk-0 width: the x "head" loaded before the tails
    tail = n_tok - ch0

    nc = bacc.Bacc("TRN2", target_bir_lowering=False, debug=False, num_devices=N_CORES)
    xt = nc.dram_tensor("xt", [K16, 128, n_tok], f16, kind="ExternalInput").ap()
    # fp8 x arrives pre-split into head/tail so each is contiguous and the
    # per-pair transfers collapse to balanced 3D APs (the DMA engine
    # cannot balance >3 genuinely-strided dims). Head: 4 pairs (chunk-0
    # chains), tail: 3 pairs (late chains).
    xt8h = nc.dram_tensor("xt8h", [NPA, 128, 2, ch0], f8, kind="ExternalInput").ap()
    xt8t = (
        nc.dram_tensor("xt8t", [NPAIR, 128, 2, tail], f8, kind="ExternalInput").ap()
        if tail
        else None
    )
    wt = nc.dram_tensor("wt", [K16, 128, D], f16, kind="ExternalInput").ap()
    wt8 = nc.dram_tensor("wt8", [NPA, 128, 2, D], f8, kind="ExternalInput").ap()
    biasw = nc.dram_tensor("biasw", [128, ET], f32, kind="ExternalInput").ap()
    y = nc.dram_tensor("y", [ET, 128, n_tok], f16, kind="ExternalOutput").ap()

    # k-units: ('f', k) = one fp16 k-tile, ('d', j) = one fp8 DoubleRow pair
    # (j indexes the 4-pair wt8; pair j covers rows 1024+256j).
    # chunk-0 chains: fp16 k0-7 + all 4 pairs = 12 PE passes.
    units_a = [("f", k) for k in range(8)] + [("d", j) for j in range(NPA)]

    with tile.TileContext(nc) as tc:
        with (
            tc.tile_pool(name="wp", bufs=1) as wp,
            tc.tile_pool(name="xp", bufs=1) as xp,
            tc.tile_pool(name="bp", bufs=1) as bp,
            # ot staging: enough bufs that DVE drains never wait on y-DMA
            # completion - on runs where the y writebacks crawl (shared-HBM
            # contention), small pools exhaust mid-run: DVE stalls on slot
            # reuse, PSUM banks stay held, and the PE goes idle (5-17us
            # observed at 14 and 28 bufs). 64 bufs stage 80% of the whole
            # output in SBUF, so a crawl episode only delays the writes,
            # never the PE; ~173KB of the ~208KB SBUF budget in total.
            tc.tile_pool(name="op", bufs=64) as op_,
            tc.tile_pool(name="pp", bufs=8, space="PSUM") as pp,
        ):
            # (w_k, x-head_k) pairs in unit order on ONE ring (sync): a
            # single priority-ordered FIFO gives each pair the FULL HBM
            # bandwidth in turn. Striping pairs across both HWDGE rings was
            # measured SLOWER every way (v4 whole-tile alternation: bursty
            # arrival, +5.7us of PE gaps; v5/v6 half-striping: pair-ready =
            # max of two jittery half-rate streams). Issue instructions cost
            # ~600-740ns on the engine with a 4-deep in-flight credit
            # window, so the head sequence also wants FEW, BIG transfers:
            # w tiles go whole (512KB) except w0, whose lo half leads so
            # chains e0-e7 can start the moment (w0-lo, xh0) lands. Rows
            # must stay >=1024B: 512B-row transfers halve DMA efficiency
            # (v5: 360 -> 206GB/s aggregate).
            # ISSUE COUNT IS A FIRST-CLASS COST: each dma_start occupies the
            # ring engine ~600-740ns and the ring holds only ~4 transfers in
            # flight (per-context completion credits), so a long stream of
            # small transfers is issue/credit-paced, not bandwidth-paced.
            # With 42 per-tensor transfers the LAST ones (x tails) did not
            # even start until ~55us and the first late chain stalled unit-
            # by-unit on them (1.2-4.9us + a PE p-state reset). 21 transfers:
            # singles while the PE still trails the DMA (units 0-3), then
            # 2-plane/6-plane combines, then 3 big tail batches.
            rings = (nc.sync, nc.scalar)
            bias_s = bp.tile([128, ET], f32, name="bias_s")
            wk, xh = [], []
            for k in range(4):
                ws = wp.tile([128, D], f16, name=f"w{k}", tag=f"w{k}")
                h = xp.tile([128, ch0], f16, name=f"xh{k}", tag=f"xh{k}")
                if k == 0:
                    nc.sync.dma_start(ws[:, 0 : D // 2], wt[k][:, 0 : D // 2])
                    nc.sync.dma_start(h[:], xt[k][:, 0:ch0])
                    nc.sync.dma_start(ws[:, D // 2 : D], wt[k][:, D // 2 : D])
                    nc.sync.dma_start(bias_s[:], biasw[:])
                else:
                    nc.sync.dma_start(ws[:], wt[k])
                    nc.sync.dma_start(h[:], xt[k][:, 0:ch0])
                wk.append(ws)
                xh.append(h)
            # units 4-7 as 2-plane pairs: one 1MB w + one 256KB xh transfer
            # per two units (bursts of 2 pairs on one ring are absorbed by
            # the 8-chain PSUM buffer; the halved issue count is pure win)
            # (rearrange the FULL dram tensor, then slice/index: rearranging
            # an already-sliced 4D AP panics in the AP library)
            wg, xhg = [], []
            wt_p = wt.rearrange("(g k) p d -> g p k d", k=2)
            xt_p = xt.rearrange("(g k) p n -> g p k n", k=2)
            for g in range(2):
                g2 = 2 + g
                w2 = wp.tile([128, 2, D], f16, name=f"wg{g}", tag=f"wg{g}")
                nc.sync.dma_start(w2[:], wt_p[g2])
                wg.append(w2)
                h2 = xp.tile([128, 2, ch0], f16, name=f"xhg{g}", tag=f"xhg{g}")
                nc.sync.dma_start(h2[:], xt_p[g2][:, :, 0:ch0])
                xhg.append(h2)
            # fp8 pairs in TWO 2-pair transfers (1.28MB w + 256KB x each):
            # bursts of 2 units are absorbed by the 8-chain PSUM buffer, and
            # every issue saved here pulls the tail transfers earlier in the
            # ring's 4-deep credit pipeline (v11 shipped 8 per-pair issues
            # and the late chains stalled 8.3us on late tails)
            w8c = wp.tile([128, NPA, 2, D], f8, name="w8c", tag="w8c")
            x8hc = xp.tile([128, NPA, 2, ch0], f8, name="x8hc", tag="x8hc")
            wt8_p = wt8.rearrange("(g j) p i d -> g p j i d", j=2)
            xt8h_p = xt8h.rearrange("(g j) p i n -> g p j i n", j=2)
            for g in range(NPA // 2):
                nc.sync.dma_start(w8c[:, 2 * g : 2 * g + 2, :, :], wt8_p[g])
                nc.sync.dma_start(x8hc[:, 2 * g : 2 * g + 2, :, :], xt8h_p[g])
            # tails next: the late chains consume them FIRST (units_l = DR
            # pairs then fp16 k ascending) and the A phase is now short
            # enough that they, not the A units, bound the transition.
            if tail:
                x8tc = xp.tile([128, NPAIR, 2, tail], f8, name="x8tc", tag="x8tc")
                nc.sync.dma_start(x8tc[:], xt8t.rearrange("j p i n -> p j i n"))
            xtl = []
            xt_r = xt.rearrange("(g k) p n -> g k p n", k=5)
            for g in range(K16 // 5):
                t = xp.tile([128, 5, tail], f16, name=f"xt{g}", tag=f"xt{g}")
                nc.sync.dma_start(
                    t[:], xt_r[g][:, :, ch0:n_tok].rearrange("k p n -> p k n")
                )
                xtl.append(t)
            # fp16 k8/k9 weights are LATE-ONLY (chunk-0 covers those rows in
            # fp8) and the LAST two units of each late chain - load them
            # after the tails. Their x heads (cols 0..511) are never read.
            w2l = wp.tile([128, 2, D], f16, name="wg2", tag="wg2")
            nc.sync.dma_start(w2l[:], wt_p[4])
            wg.append(w2l)

            def w_ap(k, e):
                if k < 4:
                    return wk[k][:, e * 128 : (e + 1) * 128]
                g, p = divmod(k - 4, 2)
                return wg[g][:, p, e * 128 : (e + 1) * 128]

            def w8_ap(j, e):
                return w8c[:, j, :, e * 128 : (e + 1) * 128]

            def x_slice(k, s0, n):
                if s0 == 0:
                    if k < 4:
                        return xh[k][:, 0:n]
                    g, p = divmod(k - 4, 2)
                    return xhg[g][:, p, 0:n]
                return xtl[k // 5][:, k % 5, s0 - ch0 : s0 - ch0 + n]

            def x8_slice(j, s0, n):
                if s0 == 0:
                    return x8hc[:, j, :, 0:n]
                # tails exist only for the late-chain pairs j=1..3
                return x8tc[:, j - 1, :, s0 - ch0 : s0 - ch0 + n]

            # PE warm-up: matmuls on a zeroed tile, no DMA dependency. Runs
            # during the DMA ramp (PE would idle anyway) and flips the HAM
            # clock gate to 8/8 before the first real matmul. memset on DVE:
            # it boots by ~4.7us and memsets in ~200ns.
            wz = bp.tile([128, 512], f16, name="wz")
            nc.vector.memset(wz[:], 0.0)
            # psw shares the chain-psum rotation: its slot is recycled by the
            # 8th chunk-0 chain, long after the warm-up finishes. Excess
            # warm-ups sit AHEAD of ready real work in the PE queue and
            # delay it (they run at p-state-ramp clock, ~430-790ns each).
            psw = pp.tile([128, 512], f32, name="psw", tag="ps")
            for _ in range(N_WARMUP):
                nc.tensor.matmul(psw[:], wz[:, 0:128], wz[:], start=True, stop=True)

            def unit_mm(ps, e, s0, n, u, start, stop):
                if u[0] == "f":
                    return nc.tensor.matmul(
                        ps[:, 0:n],
                        w_ap(u[1], e),
                        x_slice(u[1], s0, n),
                        start=start,
                        stop=stop,
                    )
                return nc.tensor.matmul(
                    ps[:, 0:n],
                    w8_ap(u[1], e),
                    x8_slice(u[1], s0, n),
                    start=start,
                    stop=stop,
                    perf_mode=DR,
                )

            def mm_chain(ps, e, s0, n, us=None):
                us = us if us is not None else units_a
                first = last = None
                for i, u in enumerate(us):
                    mm = unit_mm(ps, e, s0, n, u, i == 0, i == len(us) - 1)
                    first = first or mm
                    last = mm
                return first, last

            prev_last = None

            def pin(first, reason):
                # keep the PE stream in emission order chain-by-chain: the
                # scheduler otherwise hoists later chains (gated on late
                # arrivals) ahead of ready work and stalls the PE
                if prev_last is not None:
                    tile.add_dep_helper(
                        first.ins, prev_last.ins, sync=False, reason=reason
                    )

            def drain(ps, e, s0, n):
                ot = op_.tile([128, n], f16, name=f"ot{e}_{s0}", tag="ot")
                nc.vector.tensor_scalar_add(ot[:], ps[:, 0:n], bias_s[:, e : e + 1])
                # y alternates the two HW rings (vector/gpsimd are NOT HW
                # DGE rings - gpsimd's qGpSimdDynamic is a software queue,
                # measured +55us). Alternating halves the writeback backlog
                # each ring's final completion-wait covers.
                rings[e % 2].dma_start(y[e][:, s0 : s0 + n], ot[:])

            # phase A: chunk-0 chains, UNPINNED so the scheduler interleaves
            # them by operand arrival during the DMA ramp. 8 psum banks keep
            # 8 chains in flight (8 x 216ns of PE work per arriving k-pair
            # matches the ~1.7us pair cadence); later e-tiles draft behind
            # the frontier on already-arrived pairs.
            a_lasts = []
            s0a, n0 = CH[0]
            for e in range(ET):
                ps = pp.tile([128, 512], f32, name=f"psa{e}", tag="ps")
                fa, la = mm_chain(ps, e, s0a, n0)
                a_lasts.append(la)
                drain(ps, e, s0a, n0)

            # chunks >= 1: all operands are resident by now; strict emission
            # order keeps the PE stream dense. DR units go FIRST so the
            # chain's stop-MM is a plain fp16 one. (Merging late drains into
            # per-e staging tiles with one y-DMA measured SLOWER - the
            # teardown semaphore storm did not shrink with transfer count.)
            # late chains: fp16 k0-9 + pairs 1-3 (rows 1280..2047), DR first
            units_l = [("d", j) for j in range(1, NPA)] + [("f", k) for k in range(K16)]
            first_late = True
            for s0, n in CH[1:]:
                for e in range(ET):
                    ps = pp.tile([128, 512], f32, name=f"ps{e}_{s0}", tag="ps")
                    ff, lf = mm_chain(ps, e, s0, n, units_l)
                    if first_late:
                        # full join on ALL chunk-0 chains: softening this to
                        # the last 8 (v8) let the scheduler shuffle the
                        # transition and measured WORSE (2.8us of transition
                        # gaps vs 1.2us, plus a bunched y-writeback tail)
                        for la in a_lasts:
                            tile.add_dep_helper(ff.ins, la.ins, sync=False, reason="A->F")
                        first_late = False
                    else:
                        pin(ff, f"chain order c{s0}e{e}")
                    prev_last = lf
                    drain(ps, e, s0, n)

    nc.compile()
    return nc


def _get_program(n_tok: int):
    if n_tok not in _PROGRAM_CACHE:
        _PROGRAM_CACHE[n_tok] = _build_program(n_tok)
    return _PROGRAM_CACHE[n_tok]


def _round_up(v: int, m: int) -> int:
    return -(-v // m) * m


def _q8(a: np.ndarray, scale: float) -> np.ndarray:
    return np.clip(a * scale, -240.0, 240.0).astype(F8)


def kernel(hidden_states, type_ids, W0, b0, W1, b1, _trace=False, _tmpdir=None):
    global LAST_RESULTS

    B, S, D_ = hidden_states.shape
    assert D_ == D
    x = np.ascontiguousarray(np.asarray(hidden_states, dtype=np.float32)).reshape(
        B * S, D
    )
    t = np.asarray(type_ids).reshape(B * S)

    idx = [np.nonzero(t == e)[0] for e in (0, 1)]
    counts = [len(i) for i in idx]
    # tokens per core: 4 cores per expert, token dim rounded to 4 (moving
    # operand - no 128 padding needed). Extremely skewed expert splits fall
    # back to multiple launches of the same program over token slices.
    N_TOK_MAX = 4096
    n_tok = max(64, _round_up(-(-max(counts) // CORES_PER_EXPERT), 4))
    n_tok = min(n_tok, N_TOK_MAX)
    cap = n_tok * CORES_PER_EXPERT
    n_launches = -(-max(counts) // cap)

    nc = _get_program(n_tok)

    wts, wt8s, biases = [], [], []
    for W, b in ((W0, b0), (W1, b1)):
        WT = np.asarray(W, dtype=np.float32).T  # [d, e]
        wts.append(
            np.ascontiguousarray((WT[:KSPLIT] * SCALE).astype(np.float16)).reshape(
                K16, 128, D
            )
        )
        # pair j, plane i, partition p <-> contraction row KSPLIT_A+256j+128i+p
        # (4 pairs from row 1024; the late chains use pairs 1-3 = rows 1280+)
        wt8s.append(
            np.ascontiguousarray(
                _q8(WT[KSPLIT_A:], SW).reshape(NPA, 2, 128, D).transpose(0, 2, 1, 3)
            )
        )
        biases.append(
            np.ascontiguousarray(
                (np.asarray(b, dtype=np.float32) * SCALE).reshape(ET, 128).T
            )
        )

    gathered = [x[idx[e]] for e in (0, 1)]  # [count_e, D] fp32

    out = np.empty((B * S, D), dtype=np.float32)
    parts = [[], []]
    for li in range(n_launches):
        in_maps = []
        for e in (0, 1):
            g = gathered[e][li * cap : (li + 1) * cap]
            if g.shape[0] < cap:
                g = np.concatenate(
                    [g, np.zeros((cap - g.shape[0], D), np.float32)], axis=0
                )
            ch0 = _chunks(n_tok)[0][1]
            for c in range(CORES_PER_EXPERT):
                chunk = g[c * n_tok : (c + 1) * n_tok]  # [n_tok, D] fp32
                ct = chunk.T  # [D, n_tok]
                xt_c = np.ascontiguousarray(ct[:KSPLIT].astype(np.float16)).reshape(
                    K16, 128, n_tok
                )
                xt8_c = (
                    _q8(ct[KSPLIT_A:], SX)
                    .reshape(NPA, 2, 128, n_tok)
                    .transpose(0, 2, 1, 3)
                )
                im = {
                    "xt": xt_c,
                    "xt8h": np.ascontiguousarray(xt8_c[:, :, :, 0:ch0]),
                    "wt": wts[e],
                    "wt8": wt8s[e],
                    "biasw": biases[e],
                }
                if ch0 < n_tok:
                    # tails only for the late-chain pairs (rows 1280+)
                    im["xt8t"] = np.ascontiguousarray(xt8_c[1:, :, :, ch0:])
                in_maps.append(im)

        res = None
        for attempt in range(3):
            try:
                res = run_bass_kernel_spmd(
                    nc, in_maps, list(range(N_CORES)), trace=_trace, tmpdir=_tmpdir
                )
                break
            except Exception:
                # transient NRT_EXEC_UNIT_UNRECOVERABLE has been observed when
                # a run starts right as a previous process tears the device down
                if attempt == 2:
                    raise
                time.sleep(10)
        LAST_RESULTS = res
        for e in (0, 1):
            parts[e].extend(
                res.results[e * CORES_PER_EXPERT + c]["y"].reshape(D, n_tok).T
                for c in range(CORES_PER_EXPERT)
            )

    inv = np.float32(1.0 / SCALE)
    for e in (0, 1):
        full_e = np.concatenate(parts[e], axis=0)[: counts[e]]
        out[idx[e]] = full_e.astype(np.float32) * inv
    return out.reshape(B, S, D)



# revision 47
# speedup vs baseline: 1.1722x; 1.0204x over previous
# Mixture-of-two-experts (modality-routed) token GEMM on 8 Trainium2 NeuronCores.
#
# v7: weights-stationary. The reference computes BOTH expert GEMMs and selects
# per token; only one GEMM per token is needed. Host partitions tokens by
# type_id (expert-dispatch): cores 0-3 carry expert-0 tokens + W0, cores 4-7
# expert-1 tokens + W1 (weights arrive as data, the per-core program is
# identical). On device each core computes y[e, tok] = W x + b with W tiles
# STATIONARY and the token dim MOVING, so PE cost scales with the actual
# per-core token count (n_tok rounded to 4) instead of 128-padded m-tiles.
# Steady state measured AT the PE roofline: 384-wide fp16 matmuls issue
# 162ns apart = 1 column/cycle at 2.37GHz with LDWEIGHTS fully hidden.
#
# Mixed precision, tiered by phase: late chains (chunk>=1, ~75% of tokens)
# run fp16 for k 0..1279 + three fp8e4m3 DoubleRow pairs (2 k-planes per PE
# cell, 2 MACs/cycle) for k 1280..2047 = 13 PE passes instead of 16; the
# DMA-ramp-limited chunk-0 chains (~25% of tokens) run fp16 k 0..1023 +
# FOUR pairs = 12 passes. SW=45.25 places max|W|*SW at 1.0 so the top e4m3
# binade is [0.5,1) with step 1/16 (SW=64 wastes half the mantissa: max
# lands mid-binade at 1.41 where the step is 1/8). Host-simulated with
# exact device semantics AND hardware-verified: rel err 1.9314e-2, inside
# the 2e-2 gate (host sim matched hardware to ~1e-6 on three configs).
# Scales: x8 = x*16, w8 = W*45.25, fp16 W pre-scaled *724 so one PSUM chain
# is consistent at 724*y; the host divides the fp16 output by 724.

import os
import sys
import time

import numpy as np
import ml_dtypes

for _p in ("/opt/trn_rl_repo", "/root/.axon_site/_ro/trn_rl_repo"):
    if os.path.isdir(_p) and _p not in sys.path:
        sys.path.insert(0, _p)

import concourse.bacc as bacc
import concourse.mybir as mybir
import concourse.tile as tile
from concourse.bass_utils import run_bass_kernel_spmd

D = 2048
ET = D // 128  # 16 output-feature tiles
K16 = 10  # fp16 k-tiles (k 0..1279), used by the chunk>=1 (late) chains
NPAIR = 3  # late-chain fp8 DoubleRow pairs (k 1280..2047, 256 rows each)
NPA = 4  # chunk-0 chains use FOUR pairs (k 1024..2047) + fp16 k 0..1023:
# 12 PE passes instead of 13 for ~25% of tokens. Host-simulated rel err
# 1.9314e-2 (vs 1.8555e-2 all-b3), still under the 2e-2 gate; the extra
# pair is a superset load (pairs 1-3 are the late-chain pairs) costing one
# 512KB weight transfer while dropping the k8/k9 x-head transfers.
KSPLIT = K16 * 128
KSPLIT_A = 2048 - NPA * 256  # fp16/fp8 boundary for chunk-0 chains (1024)
SX = 16.0  # fp8 scale on x
SW = 45.25  # fp8 scale on W (max|W|*SW = 1.0: top binade step 1/16)
SCALE = SX * SW  # PSUM carries SCALE*y; host divides it out
N_CORES = 8
CORES_PER_EXPERT = 4
N_WARMUP = 6  # PE warm-up matmuls bridging PE-boot (~7.5us) to first-
# operand-ready (~10.4-10.7us). Measured: DMA issue instructions serialize
# at ~600-740ns each with a 4-deep credit window, so the first (w0lo, xh0)
# pair cannot land earlier no matter how transfers are sized or ordered;
# 6 zero-MMs at p-state-ramp clock (788+5*427ns) end right there. 8 was
# also tried: the post-warm-up wait just moved later (DMA jitter) and it
# measured worse.
F8 = ml_dtypes.float8_e4m3fn

_PROGRAM_CACHE: dict[int, object] = {}
LAST_RESULTS = None  # BassKernelResults of the most recent launch (for profiling)


def _chunks(n_tok: int):
    """Split the token dim into <=512-wide chunks, 4-aligned boundaries.

    Chunk 0 is full 512 wide when possible: during the DMA ramp the 8
    in-flight chunk-0 chains then expose 8*216ns of PE work per arriving
    (w_k, xh_k) pair, matching the ~1.7us pair cadence."""
    if n_tok <= 512:
        return [(0, n_tok)]
    rest = n_tok - 512
    nch = -(-rest // 512)
    base = rest // nch // 4 * 4
    sizes = [base] * nch
    i = 0
    while 512 + sum(sizes) < n_tok:
        sizes[i] += 4
        i = (i + 1) % nch
    sizes = [512] + sizes
    assert 512 + rest == n_tok and all(s <= 512 for s in sizes)
    out, s0 = [], 0
    for s in sizes:
        out.append((s0, s))
        s0 += s
    return out


def _build_program(n_tok: int):
    """One NeuronCore program: y[e, tok] = SCALE * (W @ x + bias), fp16 out."""
    assert n_tok % 4 == 0
    f16 = mybir.dt.float16
    f32 = mybir.dt.float32
    f8 = mybir.dt.float8e4
    DR = mybir.MatmulPerfMode.DoubleRow

    CH = _chunks(n_tok)
    ch0 = CH[0][1]  # chunk-0 width: the x "head" loaded before the tails
    tail = n_tok - ch0

    nc = bacc.Bacc("TRN2", target_bir_lowering=False, debug=False, num_devices=N_CORES)
    xt = nc.dram_tensor("xt", [K16, 128, n_tok], f16, kind="ExternalInput").ap()
    # fp8 x arrives pre-split into head/tail so each is contiguous and the
    # per-pair transfers collapse to balanced 3D APs (the DMA engine
    # cannot balance >3 genuinely-strided dims). Head: 4 pairs (chunk-0
    # chains), tail: 3 pairs (late chains).
    xt8h = nc.dram_tensor("xt8h", [NPA, 128, 2, ch0], f8, kind="ExternalInput").ap()
    xt8t = (
        nc.dram_tensor("xt8t", [NPAIR, 128, 2, tail], f8, kind="ExternalInput").ap()
        if tail
        else None
    )
    wt = nc.dram_tensor("wt", [K16, 128, D], f16, kind="ExternalInput").ap()
    wt8 = nc.dram_tensor("wt8", [NPA, 128, 2, D], f8, kind="ExternalInput").ap()
    biasw = nc.dram_tensor("biasw", [128, ET], f32, kind="ExternalInput").ap()
    y = nc.dram_tensor("y", [ET, 128, n_tok], f16, kind="ExternalOutput").ap()

    # k-units: ('f', k) = one fp16 k-tile, ('d', j) = one fp8 DoubleRow pair
    # (j indexes the 4-pair wt8; pair j covers rows 1024+256j).
    # chunk-0 chains: fp16 k0-7 + all 4 pairs = 12 PE passes.
    units_a = [("f", k) for k in range(8)] + [("d", j) for j in range(NPA)]

    with tile.TileContext(nc) as tc:
        with (
            tc.tile_pool(name="wp", bufs=1) as wp,
            tc.tile_pool(name="xp", bufs=1) as xp,
            tc.tile_pool(name="bp", bufs=1) as bp,
            # ot staging: enough bufs that DVE drains never wait on y-DMA
            # completion - on runs where the y writebacks crawl (shared-HBM
            # contention), small pools exhaust mid-run: DVE stalls on slot
            # reuse, PSUM banks stay held, and the PE goes idle (5-17us
            # observed at 14 and 28 bufs). 64 bufs stage 80% of the whole
            # output in SBUF, so a crawl episode only delays the writes,
            # never the PE; ~173KB of the ~208KB SBUF budget in total.
            tc.tile_pool(name="op", bufs=64) as op_,
            tc.tile_pool(name="pp", bufs=8, space="PSUM") as pp,
        ):
            # (w_k, x-head_k) pairs in unit order on ONE ring (sync): a
            # single priority-ordered FIFO gives each pair the FULL HBM
            # bandwidth in turn. Striping pairs across both HWDGE rings was
            # measured SLOWER every way (v4 whole-tile alternation: bursty
            # arrival, +5.7us of PE gaps; v5/v6 half-striping: pair-ready =
            # max of two jittery half-rate streams). Issue instructions cost
            # ~600-740ns on the engine with a 4-deep in-flight credit
            # window, so the head sequence also wants FEW, BIG transfers:
            # w tiles go whole (512KB) except w0, whose lo half leads so
            # chains e0-e7 can start the moment (w0-lo, xh0) lands. Rows
            # must stay >=1024B: 512B-row transfers halve DMA efficiency
            # (v5: 360 -> 206GB/s aggregate).
            # ISSUE COUNT IS A FIRST-CLASS COST: each dma_start occupies the
            # ring engine ~600-740ns and the ring holds only ~4 transfers in
            # flight (per-context completion credits), so a long stream of
            # small transfers is issue/credit-paced, not bandwidth-paced.
            # With 42 per-tensor transfers the LAST ones (x tails) did not
            # even start until ~55us and the first late chain stalled unit-
            # by-unit on them (1.2-4.9us + a PE p-state reset). 21 transfers:
            # singles while the PE still trails the DMA (units 0-3), then
            # 2-plane/6-plane combines, then 3 big tail batches.
            rings = (nc.sync, nc.scalar)
            bias_s = bp.tile([128, ET], f32, name="bias_s")
            wk, xh = [], []
            for k in range(4):
                ws = wp.tile([128, D], f16, name=f"w{k}", tag=f"w{k}")
                h = xp.tile([128, ch0], f16, name=f"xh{k}", tag=f"xh{k}")
                if k == 0:
                    nc.sync.dma_start(ws[:, 0 : D // 2], wt[k][:, 0 : D // 2])
                    nc.sync.dma_start(h[:], xt[k][:, 0:ch0])
                    nc.sync.dma_start(ws[:, D // 2 : D], wt[k][:, D // 2 : D])
                    nc.sync.dma_start(bias_s[:], biasw[:])
                else:
                    nc.sync.dma_start(ws[:], wt[k])
                    nc.sync.dma_start(h[:], xt[k][:, 0:ch0])
                wk.append(ws)
                xh.append(h)
            # units 4-7 as 2-plane pairs: one 1MB w + one 256KB xh transfer
            # per two units (bursts of 2 pairs on one ring are absorbed by
            # the 8-chain PSUM buffer; the halved issue count is pure win)
            # (rearrange the FULL dram tensor, then slice/index: rearranging
            # an already-sliced 4D AP panics in the AP library)
            wg, xhg = [], []
            wt_p = wt.rearrange("(g k) p d -> g p k d", k=2)
            xt_p = xt.rearrange("(g k) p n -> g p k n", k=2)
            for g in range(2):
                g2 = 2 + g
                w2 = wp.tile([128, 2, D], f16, name=f"wg{g}", tag=f"wg{g}")
                nc.sync.dma_start(w2[:], wt_p[g2])
                wg.append(w2)
                h2 = xp.tile([128, 2, ch0], f16, name=f"xhg{g}", tag=f"xhg{g}")
                nc.sync.dma_start(h2[:], xt_p[g2][:, :, 0:ch0])
                xhg.append(h2)
            # fp8 pairs in TWO 2-pair transfers (1.28MB w + 256KB x each):
            # bursts of 2 units are absorbed by the 8-chain PSUM buffer, and
            # every issue saved here pulls the tail transfers earlier in the
            # ring's 4-deep credit pipeline (v11 shipped 8 per-pair issues
            # and the late chains stalled 8.3us on late tails)
            w8c = wp.tile([128, NPA, 2, D], f8, name="w8c", tag="w8c")
            x8hc = xp.tile([128, NPA, 2, ch0], f8, name="x8hc", tag="x8hc")
            wt8_p = wt8.rearrange("(g j) p i d -> g p j i d", j=2)
            xt8h_p = xt8h.rearrange("(g j) p i n -> g p j i n", j=2)
            for g in range(NPA // 2):
                nc.sync.dma_start(w8c[:, 2 * g : 2 * g + 2, :, :], wt8_p[g])
                nc.sync.dma_start(x8hc[:, 2 * g : 2 * g + 2, :, :], xt8h_p[g])
            # tails next: the late chains consume them FIRST (units_l = DR
            # pairs then fp16 k ascending) and the A phase is now short
            # enough that they, not the A units, bound the transition.
            if tail:
                x8tc = xp.tile([128, NPAIR, 2, tail], f8, name="x8tc", tag="x8tc")
                nc.sync.dma_start(x8tc[:], xt8t.rearrange("j p i n -> p j i n"))
            xtl = []
            xt_r = xt.rearrange("(g k) p n -> g k p n", k=5)
            for g in range(K16 // 5):
                t = xp.tile([128, 5, tail], f16, name=f"xt{g}", tag=f"xt{g}")
                nc.sync.dma_start(
                    t[:], xt_r[g][:, :, ch0:n_tok].rearrange("k p n -> p k n")
                )
                xtl.append(t)
            # fp16 k8/k9 weights are LATE-ONLY (chunk-0 covers those rows in
            # fp8) and the LAST two units of each late chain - load them
            # after the tails, split per-tile so k8 (consumed at unit 11)
            # lands ~1.4us before k9 (unit 12). Nothing queues behind them,
            # so the extra issue is free. Their x heads are never read.
            w2l = wp.tile([128, 2, D], f16, name="wg2", tag="wg2")
            nc.sync.dma_start(w2l[:, 0, :], wt[8])
            nc.sync.dma_start(w2l[:, 1, :], wt[9])
            wg.append(w2l)

            def w_ap(k, e):
                if k < 4:
                    return wk[k][:, e * 128 : (e + 1) * 128]
                g, p = divmod(k - 4, 2)
                return wg[g][:, p, e * 128 : (e + 1) * 128]

            def w8_ap(j, e):
                return w8c[:, j, :, e * 128 : (e + 1) * 128]

            def x_slice(k, s0, n):
                if s0 == 0:
                    if k < 4:
                        return xh[k][:, 0:n]
                    g, p = divmod(k - 4, 2)
                    return xhg[g][:, p, 0:n]
                return xtl[k // 5][:, k % 5, s0 - ch0 : s0 - ch0 + n]

            def x8_slice(j, s0, n):
                if s0 == 0:
                    return x8hc[:, j, :, 0:n]
                # tails exist only for the late-chain pairs j=1..3
                return x8tc[:, j - 1, :, s0 - ch0 : s0 - ch0 + n]

            # PE warm-up: matmuls on a zeroed tile, no DMA dependency. Runs
            # during the DMA ramp (PE would idle anyway) and flips the HAM
            # clock gate to 8/8 before the first real matmul. memset on DVE:
            # it boots by ~4.7us and memsets in ~200ns.
            wz = bp.tile([128, 512], f16, name="wz")
            nc.vector.memset(wz[:], 0.0)
            # psw shares the chain-psum rotation: its slot is recycled by the
            # 8th chunk-0 chain, long after the warm-up finishes. Excess
            # warm-ups sit AHEAD of ready real work in the PE queue and
            # delay it (they run at p-state-ramp clock, ~430-790ns each).
            psw = pp.tile([128, 512], f32, name="psw", tag="ps")
            for _ in range(N_WARMUP):
                nc.tensor.matmul(psw[:], wz[:, 0:128], wz[:], start=True, stop=True)

            def unit_mm(ps, e, s0, n, u, start, stop):
                if u[0] == "f":
                    return nc.tensor.matmul(
                        ps[:, 0:n],
                        w_ap(u[1], e),
                        x_slice(u[1], s0, n),
                        start=start,
                        stop=stop,
                    )
                return nc.tensor.matmul(
                    ps[:, 0:n],
                    w8_ap(u[1], e),
                    x8_slice(u[1], s0, n),
                    start=start,
                    stop=stop,
                    perf_mode=DR,
                )

            def mm_chain(ps, e, s0, n, us=None):
                us = us if us is not None else units_a
                first = last = None
                for i, u in enumerate(us):
                    mm = unit_mm(ps, e, s0, n, u, i == 0, i == len(us) - 1)
                    first = first or mm
                    last = mm
                return first, last

            prev_last = None

            def pin(first, reason):
                # keep the PE stream in emission order chain-by-chain: the
                # scheduler otherwise hoists later chains (gated on late
                # arrivals) ahead of ready work and stalls the PE
                if prev_last is not None:
                    tile.add_dep_helper(
                        first.ins, prev_last.ins, sync=False, reason=reason
                    )

            def drain(ps, e, s0, n):
                ot = op_.tile([128, n], f16, name=f"ot{e}_{s0}", tag="ot")
                nc.vector.tensor_scalar_add(ot[:], ps[:, 0:n], bias_s[:, e : e + 1])
                # y alternates the two HW rings (vector/gpsimd are NOT HW
                # DGE rings - gpsimd's qGpSimdDynamic is a software queue,
                # measured +55us). Alternating halves the writeback backlog
                # each ring's final completion-wait covers.
                rings[e % 2].dma_start(y[e][:, s0 : s0 + n], ot[:])

            # phase A: chunk-0 chains, UNPINNED so the scheduler interleaves
            # them by operand arrival during the DMA ramp. 8 psum banks keep
            # 8 chains in flight (8 x 216ns of PE work per arriving k-pair
            # matches the ~1.7us pair cadence); later e-tiles draft behind
            # the frontier on already-arrived pairs.
            a_lasts = []
            s0a, n0 = CH[0]
            for e in range(ET):
                ps = pp.tile([128, 512], f32, name=f"psa{e}", tag="ps")
                fa, la = mm_chain(ps, e, s0a, n0)
                a_lasts.append(la)
                drain(ps, e, s0a, n0)

            # chunks >= 1: all operands are resident by now; strict emission
            # order keeps the PE stream dense. DR units go FIRST so the
            # chain's stop-MM is a plain fp16 one. (Merging late drains into
            # per-e staging tiles with one y-DMA measured SLOWER - the
            # teardown semaphore storm did not shrink with transfer count.)
            # late chains: fp16 k0-9 + pairs 1-3 (rows 1280..2047), DR first
            units_l = [("d", j) for j in range(1, NPA)] + [("f", k) for k in range(K16)]
            first_late = True
            for s0, n in CH[1:]:
                for e in range(ET):
                    ps = pp.tile([128, 512], f32, name=f"ps{e}_{s0}", tag="ps")
                    ff, lf = mm_chain(ps, e, s0, n, units_l)
                    if first_late:
                        # full join on ALL chunk-0 chains: softening this to
                        # the last 8 (v8) let the scheduler shuffle the
                        # transition and measured WORSE (2.8us of transition
                        # gaps vs 1.2us, plus a bunched y-writeback tail)
                        for la in a_lasts:
                            tile.add_dep_helper(ff.ins, la.ins, sync=False, reason="A->F")
                        first_late = False
                    else:
                        pin(ff, f"chain order c{s0}e{e}")
                    prev_last = lf
                    drain(ps, e, s0, n)

    nc.compile()
    return nc


def _get_program(n_tok: int):
    if n_tok not in _PROGRAM_CACHE:
        _PROGRAM_CACHE[n_tok] = _build_program(n_tok)
    return _PROGRAM_CACHE[n_tok]


def _round_up(v: int, m: int) -> int:
    return -(-v // m) * m


def _q8(a: np.ndarray, scale: float) -> np.ndarray:
    return np.clip(a * scale, -240.0, 240.0).astype(F8)


def kernel(hidden_states, type_ids, W0, b0, W1, b1, _trace=False, _tmpdir=None):
    global LAST_RESULTS

    B, S, D_ = hidden_states.shape
    assert D_ == D
    x = np.ascontiguousarray(np.asarray(hidden_states, dtype=np.float32)).reshape(
        B * S, D
    )
    t = np.asarray(type_ids).reshape(B * S)

    idx = [np.nonzero(t == e)[0] for e in (0, 1)]
    counts = [len(i) for i in idx]
    # tokens per core: 4 cores per expert, token dim rounded to 4 (moving
    # operand - no 128 padding needed). Extremely skewed expert splits fall
    # back to multiple launches of the same program over token slices.
    N_TOK_MAX = 4096
    n_tok = max(64, _round_up(-(-max(counts) // CORES_PER_EXPERT), 4))
    n_tok = min(n_tok, N_TOK_MAX)
    cap = n_tok * CORES_PER_EXPERT
    n_launches = -(-max(counts) // cap)

    nc = _get_program(n_tok)

    wts, wt8s, biases = [], [], []
    for W, b in ((W0, b0), (W1, b1)):
        WT = np.asarray(W, dtype=np.float32).T  # [d, e]
        wts.append(
            np.ascontiguousarray((WT[:KSPLIT] * SCALE).astype(np.float16)).reshape(
                K16, 128, D
            )
        )
        # pair j, plane i, partition p <-> contraction row KSPLIT_A+256j+128i+p
        # (4 pairs from row 1024; the late chains use pairs 1-3 = rows 1280+)
        wt8s.append(
            np.ascontiguousarray(
                _q8(WT[KSPLIT_A:], SW).reshape(NPA, 2, 128, D).transpose(0, 2, 1, 3)
            )
        )
        biases.append(
            np.ascontiguousarray(
                (np.asarray(b, dtype=np.float32) * SCALE).reshape(ET, 128).T
            )
        )

    gathered = [x[idx[e]] for e in (0, 1)]  # [count_e, D] fp32

    out = np.empty((B * S, D), dtype=np.float32)
    parts = [[], []]
    for li in range(n_launches):
        in_maps = []
        for e in (0, 1):
            g = gathered[e][li * cap : (li + 1) * cap]
            if g.shape[0] < cap:
                g = np.concatenate(
                    [g, np.zeros((cap - g.shape[0], D), np.float32)], axis=0
                )
            ch0 = _chunks(n_tok)[0][1]
            for c in range(CORES_PER_EXPERT):
                chunk = g[c * n_tok : (c + 1) * n_tok]  # [n_tok, D] fp32
                ct = chunk.T  # [D, n_tok]
                xt_c = np.ascontiguousarray(ct[:KSPLIT].astype(np.float16)).reshape(
                    K16, 128, n_tok
                )
                xt8_c = (
                    _q8(ct[KSPLIT_A:], SX)
                    .reshape(NPA, 2, 128, n_tok)
                    .transpose(0, 2, 1, 3)
                )
                im = {
                    "xt": xt_c,
                    "xt8h": np.ascontiguousarray(xt8_c[:, :, :, 0:ch0]),
                    "wt": wts[e],
                    "wt8": wt8s[e],
                    "biasw": biases[e],
                }
                if ch0 < n_tok:
                    # tails only for the late-chain pairs (rows 1280+)
                    im["xt8t"] = np.ascontiguousarray(xt8_c[1:, :, :, ch0:])
                in_maps.append(im)

        res = None
        for attempt in range(3):
            try:
                res = run_bass_kernel_spmd(
                    nc, in_maps, list(range(N_CORES)), trace=_trace, tmpdir=_tmpdir
                )
                break
            except Exception:
                # transient NRT_EXEC_UNIT_UNRECOVERABLE has been observed when
                # a run starts right as a previous process tears the device down
                if attempt == 2:
                    raise
                time.sleep(10)
        LAST_RESULTS = res
        for e in (0, 1):
            parts[e].extend(
                res.results[e * CORES_PER_EXPERT + c]["y"].reshape(D, n_tok).T
                for c in range(CORES_PER_EXPERT)
            )

    inv = np.float32(1.0 / SCALE)
    for e in (0, 1):
        full_e = np.concatenate(parts[e], axis=0)[: counts[e]]
        out[idx[e]] = full_e.astype(np.float32) * inv
    return out.reshape(B, S, D)

